# revision 38
# baseline (speedup 1.0000x reference)
"""CGGR loss kernel for 8 TRN2 NeuronCores.

Strategy (data-parallel over the flattened token axis):
  - Each core gets 512 of the 4096 token rows (full vocab, f32).
  - On-device streaming pass over the (512, 50257) shard, per 4096-col chunk:
      * DVE tensor_scalar (copy->bf16 lb) with f32 max accum  -> chunk maxes
      * ACT exp (bf16 e out) with f32 sum accum               -> sum(exp(l))
      * DVE tensor_tensor e*lb -> scr (bf16), then
        DVE tensor_scalar (scr, add accum)                    -> sum(exp(l)*l)
    Per-chunk DVE cost 5.5us < per-chunk DMA cost 5.9us, ACT 3.6us
    -> the kernel is DMA(HBM)-bound at ~287us/core.
  - Host epilogue (O(N) + one 4096-wide window gather per token):
      exact top-2 logits from chunk maxes + argmax-chunk window rescan,
      logsumexp / CE loss / entropy / margin / difficulty, global top-k
      threshold, masked mean.
"""

import numpy as np

B, S, V = 2, 2048, 50257
N = B * S                    # 4096 tokens
NCORES = 8
TPC = N // NCORES            # 512 tokens per core
P = 128
NPT = TPC // P               # 4 partition tiles per core
DMA_F = 4096                 # vocab elems per DMA chunk
NDC = (V + DMA_F - 1) // DMA_F          # 13 DMA chunks (12 full + 1105)
MAXC = 2048                  # chunk-max granularity (legacy variants)
NMC = (V + MAXC - 1) // MAXC            # 25 max chunks
OUTW = 10 * NDC              # 130 output stats per token

MIN_TOKENS_RATIO = 0.25
WARMUP_STEPS = 1000
THRESHOLD_SENSITIVITY = 0.5

# delta variant: chunks [0, H_EXACT) use fused STT for sum(e*l); the rest
# use a second ACT exp pass at scale (1+DELTA) and finite-difference on host.
H_EXACT = 4
DELTA = 4e-3

# v3 engine assignment per partition-tile (chunk indices 0..NDC-1):
#   POOL_SET: chunks whose max-reduce runs on the Pool (GPSIMD) engine
#   STT_SET:  chunks whose sum(e*l) runs on DVE via STT (exact);
#             the rest use the ACT delta-exp finite-difference pass
POOL_SET = frozenset({2, 6, 10})
STT_SET = frozenset({0, 3, 4, 7, 8, 11})

# v4: per-pt chunk sets whose sum(e*l) runs on DVE via STT (19 of 52
# chunk-instances; the rest use the ACT delta-exp pass). Spread so DVE/ACT
# load interleaves within each partition tile.
V4_STT = [
    frozenset({0, 1, 2, 3, 4}),
    frozenset({0, 1, 2, 3, 4}),
    frozenset({0, 1, 2, 3}),
    frozenset({0, 1, 2, 3, 4}),
]

# v5: chunk-max layout (host window rescan for top-2).
#   V5_STT[pt]: chunks whose sum(e*l) runs on DVE STT (rest: ACT delta-exp)
#   V5_PMAX[pt]: chunks whose max is pre-folded 4096->2048 on Pool (GPSIMD)
#                before the DVE max-accum (tail chunk 12 never folded)
V5_STT = [
    frozenset({0, 2, 4, 6, 8, 10}),
    frozenset({1, 3, 5, 7, 9, 11}),
    frozenset({0, 2, 4, 6, 8, 10}),
    frozenset({1, 3, 5, 7, 9, 11}),
]
V5_PMAX = [
    frozenset({1, 3, 5, 7, 9}),
    frozenset({0, 2, 4, 6, 8}),
    frozenset({1, 3, 5, 7, 9}),
    frozenset({0, 2, 4, 6}),
]

# v6: Pool (GPSIMD) computes sum(e*l) for V6_POOL chunks: in-place product
# e*lb on Pool, two fold-adds 4096->1024, then a DVE 1024-wide sum-accum.
# V6_STT chunks use DVE STT; the rest use the ACT delta-exp pass.
V6_POOL = frozenset({1, 4, 7, 10})
V6_STT = frozenset({0, 6})

_compiled = None
LAST_RESULTS = None          # BassKernelResults of the most recent device run


def _build(reps=1, variant="v2", dma_f=DMA_F, lp_bufs=3, maxc=MAXC,
           h_exact=H_EXACT, ob=2, loop=False):
    import concourse.bacc as bacc
    import concourse.tile as tile
    import concourse.mybir as mybir

    nc = bacc.Bacc("TRN2", target_bir_lowering=False, debug=False,
                   num_devices=NCORES)
    f32 = mybir.dt.float32
    bf16 = mybir.dt.bfloat16
    logits = nc.dram_tensor("logits", [TPC, V], f32, kind="ExternalInput")
    out = nc.dram_tensor("out", [NPT, P, OUTW], f32, kind="ExternalOutput")

    if variant.startswith("mi_"):
        return _build_micro(nc, tile, mybir, reps, variant, logits, out, loop)
    ndc = (V + dma_f - 1) // dma_f
    with tile.TileContext(nc) as tc:
        with (
            tc.tile_pool(name="lp", bufs=lp_bufs) as lp,
            tc.tile_pool(name="lbp", bufs=ob) as lbp,
            tc.tile_pool(name="ep", bufs=ob) as ep,
            tc.tile_pool(name="sp", bufs=ob) as sp,
            tc.tile_pool(name="sp2", bufs=2) as sp2,
            tc.tile_pool(name="hp", bufs=2) as hp,
            tc.tile_pool(name="accp", bufs=2) as accp,
        ):
            cw_cap = 64 if variant in ("v2dma", "v2dma2q") else None

            def do_pt_v2(pt):
                # acc layout: [0:ndc]=chunk max, [ndc:2ndc]=sum(e),
                #             [2ndc:3ndc]=sum(e*l)
                acc = accp.tile([P, 3 * ndc], f32, tag="acc")
                for dc in range(ndc):
                    w = min(dma_f, V - dc * dma_f)
                    wc = w if cw_cap is None else cw_cap
                    l = lp.tile([P, dma_f], f32, tag="l")
                    dma_eng = (nc.gpsimd if variant == "v2dma2q" and dc % 2
                               else nc.sync)
                    dma_eng.dma_start(
                        l[:, :w],
                        logits[pt * P:(pt + 1) * P,
                               dc * dma_f:dc * dma_f + w],
                    )
                    lb = lbp.tile([P, dma_f], bf16, tag="lb")
                    nc.vector.tensor_scalar(
                        out=lb[:, :wc], in0=l[:, :wc],
                        scalar1=0.0, scalar2=None,
                        op0=mybir.AluOpType.add, op1=mybir.AluOpType.max,
                        accum_out=acc[:, dc:dc + 1],
                    )
                    e = ep.tile([P, dma_f], bf16, tag="e")
                    nc.scalar.activation(
                        out=e[:, :wc], in_=l[:, :wc],
                        func=mybir.ActivationFunctionType.Exp,
                        accum_out=acc[:, ndc + dc:ndc + dc + 1],
                    )
                    scr = sp.tile([P, dma_f], bf16, tag="scr")
                    nc.vector.tensor_tensor(
                        out=scr[:, :wc], in0=e[:, :wc], in1=lb[:, :wc],
                        op=mybir.AluOpType.mult,
                    )
                    nc.vector.tensor_scalar(
                        out=scr[:, :wc], in0=scr[:, :wc],
                        scalar1=0.0, scalar2=None,
                        op0=mybir.AluOpType.add, op1=mybir.AluOpType.add,
                        accum_out=acc[:, 2 * ndc + dc:2 * ndc + dc + 1],
                    )
                nc.sync.dma_start(out[pt, :, 0:3 * ndc], acc[:])

            def do_pt_delta3(pt, stt_of_pt=None):
                acc_m8 = accp.tile([P, 8 * ndc], f32, tag="acc_m8")
                acc_se = accp.tile([P, ndc], f32, tag="acc_se")
                acc_sx = accp.tile([P, ndc], f32, tag="acc_sx")
                for dc in range(ndc):
                    w = min(dma_f, V - dc * dma_f)
                    l = lp.tile([P, dma_f], f32, tag="l")
                    nc.sync.dma_start(
                        l[:, :w],
                        logits[pt * P:(pt + 1) * P,
                               dc * dma_f:dc * dma_f + w],
                    )
                    nc.vector.max(
                        out=acc_m8[:, dc * 8:(dc + 1) * 8],
                        in_=l[:, :w])
                    e = ep.tile([P, dma_f], bf16, tag="e")
                    nc.scalar.activation(
                        out=e[:, :w], in_=l[:, :w],
                        func=mybir.ActivationFunctionType.Exp,
                        accum_out=acc_se[:, dc:dc + 1],
                    )
                    is_stt = (dc in stt_of_pt if stt_of_pt is not None
                              else dc < h_exact)
                    if is_stt:
                        scr = sp2.tile([P, dma_f], bf16, tag="scr2")
                        nc.vector.scalar_tensor_tensor(
                            out=scr[:, :w], in0=e[:, :w], scalar=1.0,
                            in1=l[:, :w],
                            op0=mybir.AluOpType.mult,
                            op1=mybir.AluOpType.mult,
                            accum_out=acc_sx[:, dc:dc + 1],
                        )
                    else:
                        scr = sp.tile([P, dma_f], bf16, tag="scr")
                        nc.scalar.activation(
                            out=scr[:, :w], in_=l[:, :w],
                            func=mybir.ActivationFunctionType.Exp,
                            scale=1.0 + DELTA,
                            accum_out=acc_sx[:, dc:dc + 1],
                        )
                nc.sync.dma_start(out[pt, :, 0:8 * ndc], acc_m8[:])
                nc.sync.dma_start(
                    out[pt, :, 8 * NDC:8 * NDC + ndc], acc_se[:])
                nc.sync.dma_start(
                    out[pt, :, 9 * NDC:9 * NDC + ndc], acc_sx[:])

            def do_pt_v4(pt):
                do_pt_delta3(pt, stt_of_pt=V4_STT[pt])

            def do_pt_v5(pt):
                # acc layout: [0:ndc]=chunk max, [ndc:2ndc]=sum(e),
                # [2ndc:3ndc]= sum(e*l) (STT chunks) | sum(e^((1+d)l)) (rest)
                acc = accp.tile([P, 3 * ndc], f32, tag="acc")
                half = dma_f // 2
                for dc in range(ndc):
                    w = min(dma_f, V - dc * dma_f)
                    l = lp.tile([P, dma_f], f32, tag="l")
                    nc.sync.dma_start(
                        l[:, :w],
                        logits[pt * P:(pt + 1) * P,
                               dc * dma_f:dc * dma_f + w],
                    )
                    macc = acc[:, dc:dc + 1]
                    junk = lbp.tile([P, dma_f], bf16, tag="lb")
                    if dc in V5_PMAX[pt] and w == dma_f:
                        h1 = hp.tile([P, half], f32, tag="h1")
                        nc.gpsimd.tensor_tensor(
                            out=h1[:], in0=l[:, :half], in1=l[:, half:w],
                            op=mybir.AluOpType.max)
                        nc.vector.tensor_scalar(
                            out=junk[:, :half], in0=h1[:],
                            scalar1=0.0, scalar2=None,
                            op0=mybir.AluOpType.add, op1=mybir.AluOpType.max,
                            accum_out=macc)
                    else:
                        nc.vector.tensor_scalar(
                            out=junk[:, :w], in0=l[:, :w],
                            scalar1=0.0, scalar2=None,
                            op0=mybir.AluOpType.add, op1=mybir.AluOpType.max,
                            accum_out=macc)
                    e = ep.tile([P, dma_f], bf16, tag="e")
                    nc.scalar.activation(
                        out=e[:, :w], in_=l[:, :w],
                        func=mybir.ActivationFunctionType.Exp,
                        accum_out=acc[:, ndc + dc:ndc + dc + 1],
                    )
                    sacc = acc[:, 2 * ndc + dc:2 * ndc + dc + 1]
                    if dc in V5_STT[pt]:
                        scr = sp2.tile([P, dma_f], bf16, tag="scr2")
                        nc.vector.scalar_tensor_tensor(
                            out=scr[:, :w], in0=e[:, :w], scalar=1.0,
                            in1=l[:, :w],
                            op0=mybir.AluOpType.mult,
                            op1=mybir.AluOpType.mult,
                            accum_out=sacc,
                        )
                    else:
                        scr = sp.tile([P, dma_f], bf16, tag="scr")
                        nc.scalar.activation(
                            out=scr[:, :w], in_=l[:, :w],
                            func=mybir.ActivationFunctionType.Exp,
                            scale=1.0 + DELTA,
                            accum_out=sacc,
                        )
                nc.sync.dma_start(out[pt, :, 0:3 * ndc], acc[:])

            def do_pt_v3(pt):
                # acc layout: [0:ndc]=chunk max, [ndc:2ndc]=sum(e),
                # [2ndc:3ndc]= sum(e*l) (STT chunks) | sum(e^((1+d)l)) (rest)
                acc = accp.tile([P, 3 * ndc], f32, tag="acc")
                for dc in range(ndc):
                    w = min(dma_f, V - dc * dma_f)
                    l = lp.tile([P, dma_f], f32, tag="l")
                    nc.sync.dma_start(
                        l[:, :w],
                        logits[pt * P:(pt + 1) * P,
                               dc * dma_f:dc * dma_f + w],
                    )
                    max_eng = nc.gpsimd if dc in POOL_SET else nc.vector
                    max_eng.tensor_reduce(
                        out=acc[:, dc:dc + 1], in_=l[:, :w],
                        op=mybir.AluOpType.max, axis=mybir.AxisListType.X)
                    e = ep.tile([P, dma_f], bf16, tag="e")
                    nc.scalar.activation(
                        out=e[:, :w], in_=l[:, :w],
                        func=mybir.ActivationFunctionType.Exp,
                        accum_out=acc[:, ndc + dc:ndc + dc + 1],
                    )
                    scr = sp.tile([P, dma_f], bf16, tag="scr")
                    sacc = acc[:, 2 * ndc + dc:2 * ndc + dc + 1]
                    if dc in STT_SET:
                        nc.vector.scalar_tensor_tensor(
                            out=scr[:, :w], in0=e[:, :w], scalar=1.0,
                            in1=l[:, :w],
                            op0=mybir.AluOpType.mult,
                            op1=mybir.AluOpType.mult,
                            accum_out=sacc,
                        )
                    else:
                        nc.scalar.activation(
                            out=scr[:, :w], in_=l[:, :w],
                            func=mybir.ActivationFunctionType.Exp,
                            scale=1.0 + DELTA,
                            accum_out=sacc,
                        )
                nc.sync.dma_start(out[pt, :, 0:3 * ndc], acc[:])

            def do_pt_v6(pt):
                # acc layout: [0:ndc]=chunk max, [ndc:2ndc]=sum(e),
                # [2ndc:3ndc]=sum(e*l) partials (exact for POOL/STT chunks,
                # delta-exp sums for the rest)
                acc = accp.tile([P, 3 * ndc], f32, tag="acc")
                half = dma_f // 2
                quart = dma_f // 4
                for dc in range(ndc):
                    w = min(dma_f, V - dc * dma_f)
                    l = lp.tile([P, dma_f], f32, tag="l")
                    nc.sync.dma_start(
                        l[:, :w],
                        logits[pt * P:(pt + 1) * P,
                               dc * dma_f:dc * dma_f + w],
                    )
                    lb = lbp.tile([P, dma_f], bf16, tag="lb")
                    nc.vector.tensor_scalar(
                        out=lb[:, :w], in0=l[:, :w],
                        scalar1=0.0, scalar2=None,
                        op0=mybir.AluOpType.add, op1=mybir.AluOpType.max,
                        accum_out=acc[:, dc:dc + 1],
                    )
                    e = ep.tile([P, dma_f], bf16, tag="e")
                    nc.scalar.activation(
                        out=e[:, :w], in_=l[:, :w],
                        func=mybir.ActivationFunctionType.Exp,
                        accum_out=acc[:, ndc + dc:ndc + dc + 1],
                    )
                    sacc = acc[:, 2 * ndc + dc:2 * ndc + dc + 1]
                    if dc in V6_POOL and w == dma_f:
                        # product in place over lb, fold 4096->2048->1024 on
                        # Pool, then 1024-wide DVE sum-accum
                        nc.gpsimd.tensor_tensor(
                            out=lb[:, :w], in0=e[:, :w], in1=lb[:, :w],
                            op=mybir.AluOpType.mult)
                        h1 = hp.tile([P, half], f32, tag="h1")
                        nc.gpsimd.tensor_tensor(
                            out=h1[:], in0=lb[:, :half], in1=lb[:, half:w],
                            op=mybir.AluOpType.add)
                        nc.gpsimd.tensor_tensor(
                            out=h1[:, :quart], in0=h1[:, :quart],
                            in1=h1[:, quart:half],
                            op=mybir.AluOpType.add)
                        scr = sp2.tile([P, dma_f], bf16, tag="scr2")
                        nc.vector.tensor_scalar(
                            out=scr[:, :quart], in0=h1[:, :quart],
                            scalar1=0.0, scalar2=None,
                            op0=mybir.AluOpType.add,
                            op1=mybir.AluOpType.add,
                            accum_out=sacc,
                        )
                    elif dc in V6_STT:
                        scr = sp2.tile([P, dma_f], bf16, tag="scr2")
                        nc.vector.scalar_tensor_tensor(
                            out=scr[:, :w], in0=e[:, :w], scalar=1.0,
                            in1=l[:, :w],
                            op0=mybir.AluOpType.mult,
                            op1=mybir.AluOpType.mult,
                            accum_out=sacc,
                        )
                    else:
                        scr = sp.tile([P, dma_f], bf16, tag="scr")
                        nc.scalar.activation(
                            out=scr[:, :w], in_=l[:, :w],
                            func=mybir.ActivationFunctionType.Exp,
                            scale=1.0 + DELTA,
                            accum_out=sacc,
                        )
                nc.sync.dma_start(out[pt, :, 0:3 * ndc], acc[:])

            do_pt = {"v2": do_pt_v2, "v2dma": do_pt_v2, "v2dma2q": do_pt_v2,
                     "v3": do_pt_v3, "v4": do_pt_v4, "v5": do_pt_v5,
                     "v6": do_pt_v6, "delta3": do_pt_delta3}[variant]

            def rep_body():
                for pt in range(NPT):
                    do_pt(pt)

            if loop and reps > 1:
                with tc.For_i(0, reps, 1):
                    rep_body()
            else:
                for _ in range(reps):
                    rep_body()

    nc.compile()
    return nc


def _build_micro(nc, tile, mybir, reps, variant, logits, out, loop=False):
    """Microbenches. Compute micros: 16 ops/rep of FD 16384 on resident
    tiles. mi_dma: one full shard sweep (52 chunk loads) per rep."""
    f32 = mybir.dt.float32
    bf16 = mybir.dt.bfloat16
    FD = 16384
    with tile.TileContext(nc) as tc:
        with (
            tc.tile_pool(name="mp", bufs=1) as mp,
            tc.tile_pool(name="lp", bufs=6) as lp,
        ):
            if variant.startswith("mi_dma"):
                two_q = variant == "mi_dma2q"
                sink = mp.tile([P, 64], bf16)

                def body():
                    for pt in range(NPT):
                        for dc in range(NDC):
                            w = min(DMA_F, V - dc * DMA_F)
                            t = lp.tile([P, DMA_F], f32, tag="l")
                            eng = nc.gpsimd if (two_q and dc % 2) else nc.sync
                            eng.dma_start(
                                t[:, :w],
                                logits[pt * P:(pt + 1) * P,
                                       dc * DMA_F:dc * DMA_F + w])
                            # tiny DVE op so every chunk is consumed
                            nc.vector.tensor_scalar(
                                out=sink[:], in0=t[:, :64], scalar1=0.0,
                                scalar2=None, op0=mybir.AluOpType.add)
                if loop and reps > 1:
                    with tc.For_i(0, reps, 1):
                        body()
                else:
                    for _ in range(reps):
                        body()
                nc.compile()
                return nc

            l = mp.tile([P, FD], f32)
            nc.sync.dma_start(l[:], logits[0:P, 0:FD])
            lb = mp.tile([P, FD], bf16)
            e = mp.tile([P, FD], bf16)
            l2 = mp.tile([P, FD], f32)
            nc.vector.tensor_scalar(out=lb[:], in0=l[:], scalar1=0.0,
                                    scalar2=None, op0=mybir.AluOpType.add)
            nc.vector.tensor_scalar(out=e[:], in0=l[:], scalar1=0.0,
                                    scalar2=None, op0=mybir.AluOpType.add)
            acc = mp.tile([P, 16], f32)
            nc.vector.memset(acc[:], 0.0)

            def one_op(j):
                a = acc[:, j % 8:j % 8 + 1]
                if variant == "mi_ts_max_acc":
                    nc.vector.tensor_scalar(
                        out=lb[:], in0=l[:], scalar1=0.0, scalar2=None,
                        op0=mybir.AluOpType.add, op1=mybir.AluOpType.max,
                        accum_out=a)
                elif variant == "mi_ts_max_acc_bf":
                    nc.vector.tensor_scalar(
                        out=e[:], in0=lb[:], scalar1=0.0, scalar2=None,
                        op0=mybir.AluOpType.add, op1=mybir.AluOpType.max,
                        accum_out=a)
                elif variant == "mi_ts_copy":
                    nc.vector.tensor_scalar(
                        out=lb[:], in0=l[:], scalar1=0.0, scalar2=None,
                        op0=mybir.AluOpType.add)
                elif variant == "mi_ts_copy_bf":
                    nc.vector.tensor_scalar(
                        out=e[:], in0=lb[:], scalar1=0.0, scalar2=None,
                        op0=mybir.AluOpType.add)
                elif variant == "mi_tt_mult":
                    nc.vector.tensor_tensor(
                        out=e[:], in0=e[:], in1=lb[:],
                        op=mybir.AluOpType.mult)
                elif variant == "mi_ts_sum_acc":
                    nc.vector.tensor_scalar(
                        out=e[:], in0=e[:], scalar1=0.0, scalar2=None,
                        op0=mybir.AluOpType.add, op1=mybir.AluOpType.add,
                        accum_out=a)
                elif variant == "mi_ts_sum_acc_f32out":
                    nc.vector.tensor_scalar(
                        out=l2[:], in0=e[:], scalar1=0.0, scalar2=None,
                        op0=mybir.AluOpType.add, op1=mybir.AluOpType.add,
                        accum_out=a)
                elif variant == "mi_stt":
                    nc.vector.scalar_tensor_tensor(
                        out=e[:], in0=e[:], scalar=1.0, in1=lb[:],
                        op0=mybir.AluOpType.mult,
                        op1=mybir.AluOpType.mult, accum_out=a)
                elif variant == "mi_max8":
                    m8 = acc[:, 0:8]
                    nc.vector.max(out=m8, in_=l[:])
                elif variant == "mi_act_exp":
                    nc.scalar.activation(
                        out=e[:], in_=l[:],
                        func=mybir.ActivationFunctionType.Exp,
                        accum_out=a)
                elif variant == "mi_gp_ts_max_acc":
                    nc.gpsimd.tensor_scalar(
                        out=lb[:], in0=l[:], scalar1=0.0, scalar2=None,
                        op0=mybir.AluOpType.add, op1=mybir.AluOpType.max,
                        accum_out=a)
                elif variant == "mi_gp_reduce_max":
                    nc.gpsimd.tensor_reduce(
                        out=a, in_=l[:], op=mybir.AluOpType.max,
                        axis=mybir.AxisListType.X)
                elif variant == "mi_dve_reduce_max":
                    nc.vector.tensor_reduce(
                        out=a, in_=l[:], op=mybir.AluOpType.max,
                        axis=mybir.AxisListType.X)
                elif variant == "mi_gp_reduce_sum":
                    nc.gpsimd.tensor_reduce(
                        out=a, in_=e[:], op=mybir.AluOpType.add,
                        axis=mybir.AxisListType.X)
                elif variant == "mi_gp_tt_max":
                    nc.gpsimd.tensor_tensor(
                        out=l2[:], in0=l[:], in1=l[:],
                        op=mybir.AluOpType.max)
                elif variant == "mi_gp_tt_add":
                    nc.gpsimd.tensor_tensor(
                        out=l2[:], in0=l[:], in1=l[:],
                        op=mybir.AluOpType.add)
                elif variant == "mi_gp_tt_max_bf":
                    nc.gpsimd.tensor_tensor(
                        out=e[:], in0=lb[:], in1=lb[:],
                        op=mybir.AluOpType.max)
                else:
                    raise ValueError(variant)

            def body():
                for j in range(16):
                    one_op(j)

            if loop and reps > 1:
                with tc.For_i(0, reps, 1):
                    body()
            else:
                for _ in range(reps):
                    body()
            nc.sync.dma_start(out[0, 0:P, 0:16], acc[:])
    nc.compile()
    return nc


PROD_VARIANT = "delta3"
PROD_KW = dict(dma_f=DMA_F, lp_bufs=6, ob=4, h_exact=5)


def _get_compiled():
    global _compiled
    if _compiled is None:
        _compiled = _build(variant=PROD_VARIANT, **PROD_KW)
    return _compiled


def _device_stats(flat_logits):
    """Run the bass kernel on 8 cores; return (N, 3*NDC) f32 stats."""
    from concourse.bass_utils import run_bass_kernel_spmd

    nc = _get_compiled()
    in_maps = [
        {"logits": np.ascontiguousarray(flat_logits[i * TPC:(i + 1) * TPC])}
        for i in range(NCORES)
    ]
    res = run_bass_kernel_spmd(nc, in_maps, list(range(NCORES)))
    global LAST_RESULTS
    LAST_RESULTS = res
    return np.concatenate(
        [res.results[i]["out"].reshape(TPC, OUTW) for i in range(NCORES)],
        axis=0,
    )


def kernel(logits, targets, step_count):
    logits = np.asarray(logits, dtype=np.float32)
    targets = np.asarray(targets).astype(np.int64)
    step = int(np.asarray(step_count))

    lf = logits.reshape(N, V)
    tf = targets.reshape(N)

    stats = _device_stats(lf)
    ar = np.arange(N)
    if PROD_VARIANT in ("v4", "delta3"):
        m8 = stats[:, :8 * NDC].astype(np.float64)     # top-8 per chunk
        se_parts = stats[:, 8 * NDC:9 * NDC].astype(np.float64)
        sx_parts = stats[:, 9 * NDC:10 * NDC].astype(np.float64)
        se = se_parts.sum(axis=1)
        # per-token STT mask: which chunk's sum(e*l) partial is exact
        pt_of_token = (ar % TPC) // P
        stt_mask = np.zeros((N, NDC), dtype=bool)
        if PROD_VARIANT == "v4":
            for pt in range(NPT):
                cols = sorted(V4_STT[pt])
                rows = pt_of_token == pt
                stt_mask[np.ix_(rows, cols)] = True
        else:
            stt_mask[:, :PROD_KW.get("h_exact", H_EXACT)] = True
        sel = np.where(stt_mask, sx_parts, 0.0).sum(axis=1) + (
            np.where(stt_mask, 0.0, sx_parts - se_parts).sum(axis=1) / DELTA
        )
        # exact top-2 logits from per-chunk top-8 candidates
        top2 = np.partition(m8, -2, axis=1)[:, -2:]
        max1 = top2[:, 1]
        max2 = top2[:, 0]
    else:
        mc = stats[:, :NDC].astype(np.float64)         # per-chunk max (f32)
        se_parts = stats[:, NDC:2 * NDC].astype(np.float64)
        sx_parts = stats[:, 2 * NDC:3 * NDC].astype(np.float64)
        se = se_parts.sum(axis=1)
        if PROD_VARIANT == "v3":
            # sum(e*l): exact STT partials for STT_SET chunks,
            # finite-difference of the two exp sums for the delta chunks
            stt = sorted(STT_SET)
            dlt = [dc for dc in range(NDC) if dc not in STT_SET]
            sel = sx_parts[:, stt].sum(axis=1) + (
                (sx_parts[:, dlt] - se_parts[:, dlt]).sum(axis=1) / DELTA
            )
        elif PROD_VARIANT == "v6":
            ex = sorted(V6_POOL | V6_STT)
            dlt = [dc for dc in range(NDC) if dc not in (V6_POOL | V6_STT)]
            sel = sx_parts[:, ex].sum(axis=1) + (
                (sx_parts[:, dlt] - se_parts[:, dlt]).sum(axis=1) / DELTA
            )
        elif PROD_VARIANT == "v5":
            pt_of_token = (ar % TPC) // P
            stt_mask = np.zeros((N, NDC), dtype=bool)
            for pt in range(NPT):
                rows = pt_of_token == pt
                stt_mask[np.ix_(rows, sorted(V5_STT[pt]))] = True
            sel = np.where(stt_mask, sx_parts, 0.0).sum(axis=1) + (
                np.where(stt_mask, 0.0, sx_parts - se_parts).sum(axis=1)
                / DELTA
            )
        else:
            sel = sx_parts.sum(axis=1)

        # exact top-2 logits: global max is the max chunk max; the runner-up
        # is either the 2nd-largest inside the argmax chunk (host window
        # rescan of the f32 logits) or the largest other chunk max.
        cstar = np.argmax(mc, axis=1)
        max1 = mc[ar, cstar]
        start = np.minimum(cstar * DMA_F, V - DMA_F)
        idx = start[:, None] + np.arange(DMA_F)[None, :]
        win = np.take_along_axis(lf, idx, axis=1)
        top2w = np.partition(win, -2, axis=1)[:, -2:].astype(np.float64)
        w2 = top2w[:, 0]                               # 2nd largest in window
        mco = mc.copy()
        mco[ar, cstar] = -np.inf
        max2 = np.maximum(w2, mco.max(axis=1))

    # epilogue in f64 (mirrors reference formulas)
    log_v = np.log(np.float32(V)).astype(np.float64)
    lse = np.log(se)
    l_tgt = lf[ar, tf].astype(np.float64)
    loss = lse - l_tgt                                 # -logp[target]
    p1 = np.exp(max1 - lse)                            # confidence
    p2 = np.exp(max2 - lse)
    margin = p1 - p2
    entropy = lse - sel / se                           # -sum p*logp
    difficulty = (entropy / log_v + (1.0 - margin) + loss / log_v) / 3.0

    progress = min(1.0, float(step) / max(1, WARMUP_STEPS))
    base_ratio = 1.0 - progress * (1.0 - MIN_TOKENS_RATIO)
    mean_conf = p1.mean()
    ratio = np.clip(
        base_ratio * (1.0 + THRESHOLD_SENSITIVITY * (0.5 - mean_conf)), 0.05, 1.0
    )
    k = int(np.clip(np.round(ratio * N), 1, N))
    thresh = np.sort(difficulty)[::-1][k - 1]
    mask = (difficulty >= thresh).astype(np.float64)
    tokens_selected = mask.sum()
    out = (loss * mask).sum() / max(tokens_selected, 1.0)
    return np.asarray(out, dtype=np.float32)
